# revision 36
# baseline (speedup 1.0000x reference)
"""Binarized dense layer (tanh(sign(x) @ sign(w) + b)) on 8 Trainium2 cores, v3.

Full input shapes (hardcoded): inputs [8192, 4096] f32, kernel [4096, 4096] f32,
bias [4096] f32 -> out [8192, 4096] f32.

Sharding: 4 batch shards x 2 output-column shards (core i -> r=i//2, c=i%2).
Per core: x [2048, 4096], w [4096, 2048], b [2048] -> y [2048, 2048].

Wire format: both x (K-major, transposed) and w are binarized ON THE HOST
to +-1.0 fp8e4 bytes (0x38 / 0xB8), exactly matching the reference
where(v<0,-1,1). All matmul operands are +-1, so PSUM accumulates the raw
integer sum S (|S| <= 4096, exact in f32); tanh needs no scale and f16
spills of half-K partials (|S'| <= 2048) are exact. The host additionally
pre-interleaves w into the DoubleRow pair layout (wire row k*128+p =
[w[k*256+p, :], w[k*256+128+p, :]]) so a K-pair DMA is a contiguous row
slice with 4KB rows, and ships y back as bf16 (~7e-4 absmax rel err vs
the 2e-2 gate) to halve output DMA.

Measured frame (per core, 2.4 GHz): 1024 DoubleRow matmuls of 512 moving
columns issue at the hw floor of ~216 ns each (~221 us); everything else
is overlap engineering: ~10 us framework preamble + first-DMA latency
covered by warm matmuls, ~9 us fixed teardown, and a ~432 ns stall every
10.79 us from an external cycle-periodic hw event.

Schedule (per core; M=16 m-tiles, KP=16 DoubleRow K-pairs, N=4 psum banks):
  - ~10us of dummy warm matmuls keep the PE HAM clock-gate busy while the
    first W pairs / X chunks stream in (PE hits 2.4 GHz before real work;
    the first DMA completion takes ~8us from issue regardless of size).
  - DMA priority order: W pair k (pair 0 in column halves so the first
    matmul gates on a 0.125MB transfer) + X chunk col-halves (cols 0:1024,
    m-tiles 0-7) interleaved for k<KH, then remaining W pairs, then
    remaining X col-halves grouped 4 chunks/DMA, h1 halves last.
    W+out DMAs issue on the sync queue, X DMAs on the gpsimd queue, and
    W pairs 1,2 on the scalar queue so their first-DMA latency overlaps
    sync's W0 and k=1,2 data lands early.
  - Each input DMA gets a cheap DVE consumer op (1 elem/partition copy)
    so the tile framework can recycle DMA-queue semaphore slots; the DVE
    queue is strict FIFO, so consumers for late-arriving DMAs are
    deferred to points where their data has certainly landed.
  - Phase 1: m-tiles 0..m_split-1 in groups of 2 (8 psum banks), k<KH,
    k-inner so the two tiles' matmuls interleave and the W-pair demand
    rate roughly matches HBM supply; partials spill to SBUF f16 (DVE and
    ACT split the copies so banks free fast).
  - Phase 2: same groups, k>=KH bank-major (each psum bank finishes its
    K-loop and evacuates - DVE partial add, ACT tanh scale=1, bf16 out
    DMA - under the next bank's matmuls, so banks recycle bubble-free).
  - m-tiles m_split..15 run single-pass full-K, also bank-major; the last
    tile DMAs out per-bank so the tail drains sooner.
"""

import sys
import types

if "/opt/trn_rl_repo" not in sys.path:
    sys.path.insert(0, "/opt/trn_rl_repo")

from contextlib import ExitStack

import numpy as np
import ml_dtypes

import concourse.bass as bass
import concourse.tile as tile
from concourse import bacc, mybir


def _ensure_ntff_hook_module():
    """The RL image's antenv lacks axon_hooks, which bass_utils imports for
    trace=True under axon. Register a functional shim in sys.modules."""
    name = "antenv.axon_hooks"
    if name in sys.modules:
        return
    try:
        import antenv
        __import__(name)
        return  # real module exists
    except ImportError:
        pass
    mod = types.ModuleType(name)
    mod._hook = None

    def set_axon_ntff_profile_hook(hook):
        mod._hook = hook

    def get_axon_ntff_profile_hook():
        if mod._hook is None:
            try:
                from trn_agent_boot.trn_boot import _ntff_profile_via_ctypes
                mod._hook = _ntff_profile_via_ctypes("/opt/axon/libaxon_pjrt.so")
            except Exception:
                return None
        return mod._hook

    mod.set_axon_ntff_profile_hook = set_axon_ntff_profile_hook
    mod.get_axon_ntff_profile_hook = get_axon_ntff_profile_hook
    sys.modules[name] = mod
    try:
        import antenv
        antenv.axon_hooks = mod
    except ImportError:
        pass


_ensure_ntff_hook_module()

from concourse.bass_utils import run_bass_kernel_spmd  # noqa: E402

F32 = mybir.dt.float32
F16 = mybir.dt.float16
BF16 = mybir.dt.bfloat16
FP8 = mybir.dt.float8e4
U32 = mybir.dt.uint32

N_CORES = 8
R_SHARDS = 4
C_SHARDS = 2

B_FULL, D_FULL, O_FULL = 8192, 4096, 4096
B_LOC = B_FULL // R_SHARDS   # 2048
O_LOC = O_FULL // C_SHARDS   # 2048

AND = mybir.AluOpType.bitwise_and
OR = mybir.AluOpType.bitwise_or
SIGN_MASK = 0x80808080
ONE_MASK = 0x38383838


def build_nc(b_loc=B_LOC, d=D_FULL, o_loc=O_LOC, bias_nonzero=False,
             out_f32=False, n_warm=120, m_split=8, kh=6):
    assert b_loc % 256 == 0 and d % 256 == 0 and o_loc % 1024 == 0
    M = b_loc // 128     # m-tiles
    KP = d // 256        # DoubleRow K-pairs
    KC = d // 128        # 128-row chunks of K
    N = o_loc // 512     # psum banks per m-tile
    BH = b_loc // 2      # x column-half (m-tiles 0..M/2-1)
    m_split = min(m_split, M)
    assert m_split % 2 == 0 and kh <= KP

    nc = bacc.Bacc("TRN2", target_bir_lowering=False, debug=False,
                   num_devices=N_CORES)
    x = nc.dram_tensor("x", [d, b_loc], FP8, kind="ExternalInput")
    # w ships pre-interleaved by the host: row k*128+p holds the K-pair k
    # DoubleRow layout [p, j, o] flattened, so a pair DMA is a plain row
    # slice with 4KB-contiguous rows.
    w = nc.dram_tensor("w", [KP * 128, 2 * o_loc], FP8, kind="ExternalInput")
    b = nc.dram_tensor("b", [o_loc], F32, kind="ExternalInput")
    out_dt = F32 if out_f32 else BF16
    y = nc.dram_tensor("y", [b_loc, o_loc], out_dt, kind="ExternalOutput")

    with tile.TileContext(nc) as tc, ExitStack() as ctx:
        singles = ctx.enter_context(tc.tile_pool(name="singles", bufs=1))
        partp = ctx.enter_context(tc.tile_pool(name="part",
                                               bufs=max(m_split, 1)))
        ostage = ctx.enter_context(tc.tile_pool(name="ost", bufs=4))
        pacc = ctx.enter_context(tc.tile_pool(name="pa", bufs=8,
                                              space="PSUM"))

        # ---- PE warmers: keep HAM busy while inputs stream in ----
        warmsrc = singles.tile([128, 128], FP8)
        nc.gpsimd.memset(warmsrc[:], 0)
        wpa = pacc.tile([128, 512], F32, tag="pa", name="warm")
        for _ in range(n_warm):
            nc.tensor.matmul(wpa[:, 0:128], warmsrc[:], warmsrc[:],
                             start=True, stop=True)

        # ---- residents ----
        xbt = singles.tile([128, KC, b_loc], FP8)
        wb = [singles.tile([128, 2, o_loc], FP8, name=f"wb{k}")
              for k in range(KP)]

        bias_bc = None
        if bias_nonzero:
            bias_bc = singles.tile([128, o_loc], F32)
            bias_ap = bass.AP(tensor=b.ap().tensor, offset=0,
                              ap=[[0, 128], [1, o_loc]])
            nc.sync.dma_start(out=bias_bc[:], in_=bias_ap)

        # ---- input DMAs, in supply-priority order ----
        # Inputs arrive pre-binarized (+-1 fp8 built on the host), so the
        # DVE no longer rewrites them. But each DMA still needs an early,
        # cheap consumer op so the tile framework can recycle its DMA-queue
        # semaphore slot (~8 per queue; a slot frees only after a consumer
        # runs). A 1-elem/partition DVE copy into a dump tile plays that
        # role. The DVE queue is strict FIFO: a consumer waiting on a late
        # DMA would block every spill/add queued behind it and stall PSUM
        # recycling. So only the phase-1-critical consumers are emitted
        # here; the rest are deferred (to points between the phase bodies
        # below) via `deferred`.
        dump = singles.tile([128, 2], FP8)

        def consume(sl_small):
            nc.vector.tensor_copy(out=dump[:, 0:1], in_=sl_small)

        deferred = []

        def dma_w_pair(k, eng=None, do_con=True, dma_splits=1):
            # whole-pair DMAs except pair 0, which arrives in column slices
            # so the very first matmul gates on a 0.125MB transfer
            eng = eng or nc.sync
            dstep = o_loc // dma_splits
            for s in range(dma_splits):
                if dma_splits == 1:
                    eng.dma_start(out=wb[k][:],
                                  in_=w[k * 128:(k + 1) * 128, :])
                else:
                    src = bass.AP(tensor=w.ap().tensor,
                                  offset=k * 128 * 2 * o_loc + s * dstep,
                                  ap=[[2 * o_loc, 128], [o_loc, 2],
                                      [1, dstep]])
                    eng.dma_start(out=wb[k][:, :, s * dstep:(s + 1) * dstep],
                                  in_=src)
                if do_con is None:        # queue holds <=8 DMAs: no consumer
                    continue
                sl_small = wb[k][:, 0:1, s * dstep:s * dstep + 1]
                if do_con:
                    consume(sl_small)
                else:
                    deferred.append(sl_small)

        def dma_x_h0(c):
            sl = xbt[:, c, 0:BH]
            nc.gpsimd.dma_start(out=sl, in_=x[c * 128:(c + 1) * 128, 0:BH])
            consume(xbt[:, c, 0:1])

        def dma_x_group(c0, nch, h, eng, do_con=None):
            # one DMA covering chunks [c0, c0+nch) cols of half h; on a
            # low-traffic queue (<=8 DMAs) no consumer is needed
            lo, hi = (0, BH) if h == 0 else (BH, b_loc)
            sl = xbt[:, c0:c0 + nch, lo:hi]
            src = bass.AP(tensor=x.ap().tensor, offset=c0 * 128 * b_loc + lo,
                          ap=[[b_loc, 128], [128 * b_loc, nch], [1, hi - lo]])
            eng.dma_start(out=sl, in_=src)
            if do_con is not None and not do_con:
                deferred.append(xbt[:, c0, lo:lo + 1])

        def flush_deferred(n):
            for sl_small in deferred[:n]:
                consume(sl_small)
            del deferred[:n]

        for k in range(kh):
            dma_w_pair(k, dma_splits=(2 if k == 0 else 1))
            dma_x_h0(2 * k)
            dma_x_h0(2 * k + 1)
        for k in range(kh, KP):
            dma_w_pair(k, do_con=False)
        for c in range(2 * kh, KC, 4):
            dma_x_group(c, 4, 0, nc.gpsimd, do_con=False)
        n_early_def = len(deferred)   # pairs kh.. + x-h0 groups
        for c in range(0, KC, 4):
            dma_x_group(c, 4, 1, nc.gpsimd, do_con=False)
        n_h1_def = len(deferred) - n_early_def

        # ---- matmul machinery ----
        def alloc_banks(m):
            return [pacc.tile([128, 512], F32, tag="pa", name=f"pa_{m}_{n}")
                    for n in range(N)]

        def mm(pa, m, k, start, stop):
            lhsT = xbt[:, 2 * k:2 * k + 2, m * 128:(m + 1) * 128]
            for n in range(N):
                nc.tensor.matmul(
                    pa[n][:], lhsT, wb[k][:, :, n * 512:(n + 1) * 512],
                    start=start, stop=stop,
                    perf_mode=mybir.MatmulPerfMode.DoubleRow)

        def finish(m, pa, part, out_banks=2):
            o = ostage.tile([128, o_loc], out_dt, tag="o", name=f"o{m}")
            nb = N // out_banks   # banks per out DMA
            for n in range(N):
                if part is not None:
                    nc.vector.tensor_tensor(out=pa[n][:], in0=pa[n][:],
                                            in1=part[:, n, :],
                                            op=mybir.AluOpType.add)
                if bias_bc is not None:
                    nc.vector.tensor_tensor(
                        out=pa[n][:], in0=pa[n][:],
                        in1=bias_bc[:, n * 512:(n + 1) * 512],
                        op=mybir.AluOpType.add)
                nc.scalar.activation(
                    out=o[:, n * 512:(n + 1) * 512], in_=pa[n][:],
                    func=mybir.ActivationFunctionType.Tanh, scale=1.0)
                if (n + 1) % nb == 0:
                    lo, hi = (n + 1 - nb) * 512, (n + 1) * 512
                    nc.sync.dma_start(out=y[m * 128:(m + 1) * 128, lo:hi],
                                      in_=o[:, lo:hi])

        # ---- phase 1: m-pair groups, k < kh, spill f16 partials ----
        parts = {}
        for g in range(m_split // 2):
            ms = (2 * g, 2 * g + 1)
            pas = {m: alloc_banks(m) for m in ms}
            for k in range(kh):
                for m in ms:
                    mm(pas[m], m, k, start=(k == 0), stop=(k == kh - 1))
            for m in ms:
                part = partp.tile([128, N, 512], F16, tag="part",
                                  name=f"part{m}")
                for n in range(N):
                    # split spills DVE/ACT so banks free in parallel
                    if n % 2 == 0:
                        nc.vector.tensor_copy(out=part[:, n, :],
                                              in_=pas[m][n][:])
                    else:
                        nc.scalar.activation(
                            out=part[:, n, :], in_=pas[m][n][:],
                            func=mybir.ActivationFunctionType.Copy)
                parts[m] = part
            # late consumers slot in behind this group's spills; their
            # DMAs have landed by now so they can't block the next group,
            # and flushing early recycles the sync queue's sem slots so
            # the last W pairs issue well before phase 2 needs them
            if g == 1:
                flush_deferred((n_early_def + 1) // 2)
            elif g == 2:
                flush_deferred(n_early_def // 2)

        def mm1(pa, m, k, n, start, stop):
            lhsT = xbt[:, 2 * k:2 * k + 2, m * 128:(m + 1) * 128]
            nc.tensor.matmul(
                pa[n][:], lhsT, wb[k][:, :, n * 512:(n + 1) * 512],
                start=start, stop=stop,
                perf_mode=mybir.MatmulPerfMode.DoubleRow)

        def evac_bank(m, pa, part, o, n, out_banks, split_last=False):
            if part is not None:
                nc.vector.tensor_tensor(out=pa[n][:], in0=pa[n][:],
                                        in1=part[:, n, :],
                                        op=mybir.AluOpType.add)
            if bias_bc is not None:
                nc.vector.tensor_tensor(
                    out=pa[n][:], in0=pa[n][:],
                    in1=bias_bc[:, n * 512:(n + 1) * 512],
                    op=mybir.AluOpType.add)
            if split_last:
                # final bank of the kernel: halve tanh+DMA so the out DMA
                # overlaps the second tanh and the tail drains sooner
                for h in range(2):
                    lo, hi = n * 512 + h * 256, n * 512 + (h + 1) * 256
                    nc.scalar.activation(
                        out=o[:, lo:hi], in_=pa[n][:, h * 256:(h + 1) * 256],
                        func=mybir.ActivationFunctionType.Tanh, scale=1.0)
                    nc.sync.dma_start(out=y[m * 128:(m + 1) * 128, lo:hi],
                                      in_=o[:, lo:hi])
                return
            nc.scalar.activation(
                out=o[:, n * 512:(n + 1) * 512], in_=pa[n][:],
                func=mybir.ActivationFunctionType.Tanh, scale=1.0)
            nb = N // out_banks
            if (n + 1) % nb == 0:
                lo, hi = (n + 1 - nb) * 512, (n + 1) * 512
                nc.sync.dma_start(out=y[m * 128:(m + 1) * 128, lo:hi],
                                  in_=o[:, lo:hi])

        # ---- phase 2: finish the split tiles (k >= kh, add partial) ----
        # bank-major: each psum bank's k-loop completes and evacuates under
        # the next bank's matmuls, so banks recycle without WAR bubbles
        for g in range(m_split // 2):
            ms = (2 * g, 2 * g + 1)
            pas = {m: alloc_banks(m) for m in ms}
            os_ = {m: ostage.tile([128, o_loc], out_dt, tag="o", name=f"o{m}")
                   for m in ms}
            for n in range(N):
                for m in ms:
                    for k in range(kh, KP):
                        mm1(pas[m], m, k, n, start=(k == kh),
                            stop=(k == KP - 1))
                for m in ms:
                    evac_bank(m, pas[m], parts[m], os_[m], n, out_banks=2)
            if g < 2:
                flush_deferred((n_h1_def + 1) // 2)  # x h1, for m-tiles 8+
        flush_deferred(len(deferred))

        # ---- remaining m-tiles: single-pass full K, bank-major ----
        for m in range(m_split, M):
            pa = alloc_banks(m)
            o = ostage.tile([128, o_loc], out_dt, tag="o", name=f"o{m}")
            ob = 4 if m == M - 1 else 2
            for n in range(N):
                for k in range(KP):
                    mm1(pa, m, k, n, start=(k == 0), stop=(k == KP - 1))
                evac_bank(m, pa, None, o, n, out_banks=ob,
                          split_last=(m == M - 1 and n == N - 1))

    nc.compile()
    return nc


_NC_CACHE = {}


def _get_nc(key, **kwargs):
    if key not in _NC_CACHE:
        _NC_CACHE[key] = build_nc(**kwargs)
    return _NC_CACHE[key]


def kernel(inputs: np.ndarray, kernel: np.ndarray, bias: np.ndarray,
           _trace: bool = False, _trace_cores=None, **_build_overrides) -> np.ndarray:
    # host-side binarize: ship {-1.0, +1.0} fp8e4 directly (0x38 / 0xB8),
    # exactly matching the reference where(v < 0, -1, 1)
    x8 = np.where(np.asarray(inputs, dtype=np.float32) < 0,
                  np.uint8(0xB8), np.uint8(0x38)).view(ml_dtypes.float8_e4m3)
    w8 = np.where(np.asarray(kernel, dtype=np.float32) < 0,
                  np.uint8(0xB8), np.uint8(0x38)).view(ml_dtypes.float8_e4m3)
    b = np.ascontiguousarray(bias, dtype=np.float32)
    assert x8.shape == (B_FULL, D_FULL) and w8.shape == (D_FULL, O_FULL)

    bias_nonzero = bool(np.any(b != 0))
    key = ("v3", bias_nonzero, tuple(sorted(_build_overrides.items())))
    nc = _get_nc(key, bias_nonzero=bias_nonzero, **_build_overrides)

    in_maps = []
    for i in range(N_CORES):
        r, c = i // C_SHARDS, i % C_SHARDS
        # pre-interleave w for DoubleRow: wire row k*128+p = [w[k*256+p],
        # w[k*256+128+p]] concatenated -> pair DMA is a contiguous row slice
        wsh = w8[:, c * O_LOC:(c + 1) * O_LOC]
        wt = wsh.reshape(D_FULL // 256, 2, 128, O_LOC).transpose(0, 2, 1, 3)
        in_maps.append({
            "x": np.ascontiguousarray(x8[r * B_LOC:(r + 1) * B_LOC, :].T),
            "w": np.ascontiguousarray(wt).reshape(D_FULL // 2, 2 * O_LOC),
            "b": np.ascontiguousarray(b[c * O_LOC:(c + 1) * O_LOC]),
        })

    res = run_bass_kernel_spmd(nc, in_maps, list(range(N_CORES)),
                               trace=_trace, trace_cores=_trace_cores)

    out = np.empty((B_FULL, O_FULL), dtype=np.float32)
    for i in range(N_CORES):
        r, c = i // C_SHARDS, i % C_SHARDS
        out[r * B_LOC:(r + 1) * B_LOC, c * O_LOC:(c + 1) * O_LOC] = \
            np.asarray(res.results[i]["y"]).astype(np.float32)

    if _trace:
        return out, res
    return out



# revision 37
# speedup vs baseline: 1.1678x; 1.1678x over previous
"""Binarized dense layer (tanh(sign(x) @ sign(w) + b)) on 8 Trainium2 cores, v3.

Full input shapes (hardcoded): inputs [8192, 4096] f32, kernel [4096, 4096] f32,
bias [4096] f32 -> out [8192, 4096] f32.

Sharding: 4 batch shards x 2 output-column shards (core i -> r=i//2, c=i%2).
Per core: x [2048, 4096], w [4096, 2048], b [2048] -> y [2048, 2048].

Wire format: both x (K-major, transposed) and w are binarized ON THE HOST
to +-1.0 fp8e4 bytes (0x38 / 0xB8), exactly matching the reference
where(v<0,-1,1). All matmul operands are +-1, so PSUM accumulates the raw
integer sum S (|S| <= 4096, exact in f32); tanh needs no scale and f16
spills of half-K partials (|S'| <= 2048) are exact. The host additionally
pre-interleaves w into the DoubleRow pair layout (wire row k*128+p =
[w[k*256+p, :], w[k*256+128+p, :]]) so a K-pair DMA is a contiguous row
slice with 4KB rows, and ships y back as bf16 (~7e-4 absmax rel err vs
the 2e-2 gate) to halve output DMA.

Measured frame (per core, 2.4 GHz): 1024 DoubleRow matmuls of 512 moving
columns issue at the hw floor of ~216 ns each (~221 us); everything else
is overlap engineering: ~10 us framework preamble + first-DMA latency
covered by warm matmuls, ~9 us fixed teardown, and a ~432 ns stall every
10.79 us from an external cycle-periodic hw event.

Schedule (per core; M=16 m-tiles, KP=16 DoubleRow K-pairs, N=4 psum banks):
  - ~10us of dummy warm matmuls keep the PE HAM clock-gate busy while the
    first W pairs / X chunks stream in (PE hits 2.4 GHz before real work;
    the first DMA completion takes ~8us from issue regardless of size).
  - DMA priority order: W pair k (pair 0 in column halves so the first
    matmul gates on a 0.125MB transfer) + X chunk col-halves (cols 0:1024,
    m-tiles 0-7) interleaved for k<KH, then remaining W pairs, then
    remaining X col-halves grouped 4 chunks/DMA, h1 halves last.
    W+out DMAs issue on the sync queue, X DMAs on the gpsimd queue, and
    W pairs 1,2 on the scalar queue so their first-DMA latency overlaps
    sync's W0 and k=1,2 data lands early.
  - Each input DMA gets a cheap DVE consumer op (1 elem/partition copy)
    so the tile framework can recycle DMA-queue semaphore slots; the DVE
    queue is strict FIFO, so consumers for late-arriving DMAs are
    deferred to points where their data has certainly landed.
  - Phase 1: m-tiles 0..m_split-1 in groups of 2 (8 psum banks), k<KH,
    k-inner so the two tiles' matmuls interleave and the W-pair demand
    rate roughly matches HBM supply; partials spill to SBUF f16 (DVE and
    ACT split the copies so banks free fast).
  - Phase 2: same groups, k>=KH bank-major (each psum bank finishes its
    K-loop and evacuates - DVE partial add, ACT tanh scale=1, bf16 out
    DMA - under the next bank's matmuls, so banks recycle bubble-free).
  - m-tiles m_split..15 run single-pass full-K, also bank-major; the last
    tile DMAs out per-bank so the tail drains sooner.
"""

import sys
import types

if "/opt/trn_rl_repo" not in sys.path:
    sys.path.insert(0, "/opt/trn_rl_repo")

from contextlib import ExitStack

import numpy as np
import ml_dtypes

import concourse.bass as bass
import concourse.tile as tile
from concourse import bacc, mybir


def _ensure_ntff_hook_module():
    """The RL image's antenv lacks axon_hooks, which bass_utils imports for
    trace=True under axon. Register a functional shim in sys.modules."""
    name = "antenv.axon_hooks"
    if name in sys.modules:
        return
    try:
        import antenv
        __import__(name)
        return  # real module exists
    except ImportError:
        pass
    mod = types.ModuleType(name)
    mod._hook = None

    def set_axon_ntff_profile_hook(hook):
        mod._hook = hook

    def get_axon_ntff_profile_hook():
        if mod._hook is None:
            try:
                from trn_agent_boot.trn_boot import _ntff_profile_via_ctypes
                mod._hook = _ntff_profile_via_ctypes("/opt/axon/libaxon_pjrt.so")
            except Exception:
                return None
        return mod._hook

    mod.set_axon_ntff_profile_hook = set_axon_ntff_profile_hook
    mod.get_axon_ntff_profile_hook = get_axon_ntff_profile_hook
    sys.modules[name] = mod
    try:
        import antenv
        antenv.axon_hooks = mod
    except ImportError:
        pass


_ensure_ntff_hook_module()

from concourse.bass_utils import run_bass_kernel_spmd  # noqa: E402

F32 = mybir.dt.float32
F16 = mybir.dt.float16
BF16 = mybir.dt.bfloat16
FP8 = mybir.dt.float8e4
U32 = mybir.dt.uint32

N_CORES = 8
R_SHARDS = 4
C_SHARDS = 2

B_FULL, D_FULL, O_FULL = 8192, 4096, 4096
B_LOC = B_FULL // R_SHARDS   # 2048
O_LOC = O_FULL // C_SHARDS   # 2048

AND = mybir.AluOpType.bitwise_and
OR = mybir.AluOpType.bitwise_or
SIGN_MASK = 0x80808080
ONE_MASK = 0x38383838


def build_nc(b_loc=B_LOC, d=D_FULL, o_loc=O_LOC, bias_nonzero=False,
             out_f32=False, n_warm=126, m_split=8, kh=6):
    assert b_loc % 256 == 0 and d % 256 == 0 and o_loc % 1024 == 0
    M = b_loc // 128     # m-tiles
    KP = d // 256        # DoubleRow K-pairs
    KC = d // 128        # 128-row chunks of K
    N = o_loc // 512     # psum banks per m-tile
    BH = b_loc // 2      # x column-half (m-tiles 0..M/2-1)
    m_split = min(m_split, M)
    assert m_split % 2 == 0 and kh <= KP

    nc = bacc.Bacc("TRN2", target_bir_lowering=False, debug=False,
                   num_devices=N_CORES)
    x = nc.dram_tensor("x", [d, b_loc], FP8, kind="ExternalInput")
    # w ships pre-interleaved by the host: row k*128+p holds the K-pair k
    # DoubleRow layout [p, j, o] flattened, so a pair DMA is a plain row
    # slice with 4KB-contiguous rows.
    w = nc.dram_tensor("w", [KP * 128, 2 * o_loc], FP8, kind="ExternalInput")
    b = nc.dram_tensor("b", [o_loc], F32, kind="ExternalInput")
    out_dt = F32 if out_f32 else BF16
    y = nc.dram_tensor("y", [b_loc, o_loc], out_dt, kind="ExternalOutput")

    with tile.TileContext(nc) as tc, ExitStack() as ctx:
        singles = ctx.enter_context(tc.tile_pool(name="singles", bufs=1))
        partp = ctx.enter_context(tc.tile_pool(name="part",
                                               bufs=max(m_split, 1)))
        ostage = ctx.enter_context(tc.tile_pool(name="ost", bufs=4))
        pacc = ctx.enter_context(tc.tile_pool(name="pa", bufs=8,
                                              space="PSUM"))

        # ---- PE warmers: keep HAM busy while inputs stream in ----
        warmsrc = singles.tile([128, 128], FP8)
        nc.gpsimd.memset(warmsrc[:], 0)
        wpa = pacc.tile([128, 512], F32, tag="pa", name="warm")
        for _ in range(n_warm):
            nc.tensor.matmul(wpa[:, 0:128], warmsrc[:], warmsrc[:],
                             start=True, stop=True)

        # ---- residents ----
        xbt = singles.tile([128, KC, b_loc], FP8)
        wb = [singles.tile([128, 2, o_loc], FP8, name=f"wb{k}")
              for k in range(KP)]

        bias_bc = None
        if bias_nonzero:
            bias_bc = singles.tile([128, o_loc], F32)
            bias_ap = bass.AP(tensor=b.ap().tensor, offset=0,
                              ap=[[0, 128], [1, o_loc]])
            nc.sync.dma_start(out=bias_bc[:], in_=bias_ap)

        # ---- input DMAs, in supply-priority order ----
        # Inputs arrive pre-binarized (+-1 fp8 built on the host), so the
        # DVE no longer rewrites them. But each DMA still needs an early,
        # cheap consumer op so the tile framework can recycle its DMA-queue
        # semaphore slot (~8 per queue; a slot frees only after a consumer
        # runs). A 1-elem/partition DVE copy into a dump tile plays that
        # role. The DVE queue is strict FIFO: a consumer waiting on a late
        # DMA would block every spill/add queued behind it and stall PSUM
        # recycling. So only the phase-1-critical consumers are emitted
        # here; the rest are deferred (to points between the phase bodies
        # below) via `deferred`.
        dump = singles.tile([128, 2], FP8)

        def consume(sl_small):
            nc.vector.tensor_copy(out=dump[:, 0:1], in_=sl_small)

        deferred = []

        def dma_w_pair(k, eng=None, do_con=True, dma_splits=1):
            # whole-pair DMAs except pair 0, which arrives in column slices
            # so the very first matmul gates on a 0.125MB transfer
            eng = eng or nc.sync
            dstep = o_loc // dma_splits
            for s in range(dma_splits):
                if dma_splits == 1:
                    eng.dma_start(out=wb[k][:],
                                  in_=w[k * 128:(k + 1) * 128, :])
                else:
                    src = bass.AP(tensor=w.ap().tensor,
                                  offset=k * 128 * 2 * o_loc + s * dstep,
                                  ap=[[2 * o_loc, 128], [o_loc, 2],
                                      [1, dstep]])
                    eng.dma_start(out=wb[k][:, :, s * dstep:(s + 1) * dstep],
                                  in_=src)
                if do_con is None:        # queue holds <=8 DMAs: no consumer
                    continue
                sl_small = wb[k][:, 0:1, s * dstep:s * dstep + 1]
                if do_con:
                    consume(sl_small)
                else:
                    deferred.append(sl_small)

        def dma_x_h0(c):
            sl = xbt[:, c, 0:BH]
            nc.gpsimd.dma_start(out=sl, in_=x[c * 128:(c + 1) * 128, 0:BH])
            consume(xbt[:, c, 0:1])

        def dma_x_group(c0, nch, h, eng, do_con=None):
            # one DMA covering chunks [c0, c0+nch) cols of half h; on a
            # low-traffic queue (<=8 DMAs) no consumer is needed
            lo, hi = (0, BH) if h == 0 else (BH, b_loc)
            sl = xbt[:, c0:c0 + nch, lo:hi]
            src = bass.AP(tensor=x.ap().tensor, offset=c0 * 128 * b_loc + lo,
                          ap=[[b_loc, 128], [128 * b_loc, nch], [1, hi - lo]])
            eng.dma_start(out=sl, in_=src)
            if do_con is not None and not do_con:
                deferred.append(xbt[:, c0, lo:lo + 1])

        def flush_deferred(n):
            for sl_small in deferred[:n]:
                consume(sl_small)
            del deferred[:n]

        for k in range(kh):
            dma_w_pair(k, dma_splits=(2 if k == 0 else 1))
            dma_x_h0(2 * k)
            dma_x_h0(2 * k + 1)
        for k in range(kh, KP):
            dma_w_pair(k, do_con=False)
        for c in range(2 * kh, KC, 4):
            dma_x_group(c, 4, 0, nc.gpsimd, do_con=False)
        n_early_def = len(deferred)   # pairs kh.. + x-h0 groups
        for c in range(0, KC, 4):
            dma_x_group(c, 4, 1, nc.gpsimd, do_con=False)
        n_h1_def = len(deferred) - n_early_def

        # ---- matmul machinery ----
        def alloc_banks(m):
            return [pacc.tile([128, 512], F32, tag="pa", name=f"pa_{m}_{n}")
                    for n in range(N)]

        def mm(pa, m, k, start, stop):
            lhsT = xbt[:, 2 * k:2 * k + 2, m * 128:(m + 1) * 128]
            for n in range(N):
                nc.tensor.matmul(
                    pa[n][:], lhsT, wb[k][:, :, n * 512:(n + 1) * 512],
                    start=start, stop=stop,
                    perf_mode=mybir.MatmulPerfMode.DoubleRow)

        def finish(m, pa, part, out_banks=2):
            o = ostage.tile([128, o_loc], out_dt, tag="o", name=f"o{m}")
            nb = N // out_banks   # banks per out DMA
            for n in range(N):
                if part is not None:
                    nc.vector.tensor_tensor(out=pa[n][:], in0=pa[n][:],
                                            in1=part[:, n, :],
                                            op=mybir.AluOpType.add)
                if bias_bc is not None:
                    nc.vector.tensor_tensor(
                        out=pa[n][:], in0=pa[n][:],
                        in1=bias_bc[:, n * 512:(n + 1) * 512],
                        op=mybir.AluOpType.add)
                nc.scalar.activation(
                    out=o[:, n * 512:(n + 1) * 512], in_=pa[n][:],
                    func=mybir.ActivationFunctionType.Tanh, scale=1.0)
                if (n + 1) % nb == 0:
                    lo, hi = (n + 1 - nb) * 512, (n + 1) * 512
                    nc.sync.dma_start(out=y[m * 128:(m + 1) * 128, lo:hi],
                                      in_=o[:, lo:hi])

        # ---- phase 1: m-pair groups, k < kh, spill f16 partials ----
        parts = {}
        for g in range(m_split // 2):
            ms = (2 * g, 2 * g + 1)
            pas = {m: alloc_banks(m) for m in ms}
            for k in range(kh):
                for m in ms:
                    mm(pas[m], m, k, start=(k == 0), stop=(k == kh - 1))
            for m in ms:
                part = partp.tile([128, N, 512], F16, tag="part",
                                  name=f"part{m}")
                for n in range(N):
                    # split spills DVE/ACT so banks free in parallel
                    if n % 2 == 0:
                        nc.vector.tensor_copy(out=part[:, n, :],
                                              in_=pas[m][n][:])
                    else:
                        nc.scalar.activation(
                            out=part[:, n, :], in_=pas[m][n][:],
                            func=mybir.ActivationFunctionType.Copy)
                parts[m] = part
            # late consumers slot in behind this group's spills; their
            # DMAs have landed by now so they can't block the next group,
            # and flushing early recycles the sync queue's sem slots so
            # the last W pairs issue well before phase 2 needs them
            if g == 1:
                flush_deferred((n_early_def + 1) // 2)
            elif g == 2:
                flush_deferred(n_early_def // 2)

        def mm1(pa, m, k, n, start, stop):
            lhsT = xbt[:, 2 * k:2 * k + 2, m * 128:(m + 1) * 128]
            nc.tensor.matmul(
                pa[n][:], lhsT, wb[k][:, :, n * 512:(n + 1) * 512],
                start=start, stop=stop,
                perf_mode=mybir.MatmulPerfMode.DoubleRow)

        def evac_bank(m, pa, part, o, n, out_banks, split_last=False):
            if part is not None:
                nc.vector.tensor_tensor(out=pa[n][:], in0=pa[n][:],
                                        in1=part[:, n, :],
                                        op=mybir.AluOpType.add)
            if bias_bc is not None:
                nc.vector.tensor_tensor(
                    out=pa[n][:], in0=pa[n][:],
                    in1=bias_bc[:, n * 512:(n + 1) * 512],
                    op=mybir.AluOpType.add)
            if split_last:
                # final bank of the kernel: halve tanh+DMA so the out DMA
                # overlaps the second tanh and the tail drains sooner
                for h in range(2):
                    lo, hi = n * 512 + h * 256, n * 512 + (h + 1) * 256
                    nc.scalar.activation(
                        out=o[:, lo:hi], in_=pa[n][:, h * 256:(h + 1) * 256],
                        func=mybir.ActivationFunctionType.Tanh, scale=1.0)
                    nc.sync.dma_start(out=y[m * 128:(m + 1) * 128, lo:hi],
                                      in_=o[:, lo:hi])
                return
            nc.scalar.activation(
                out=o[:, n * 512:(n + 1) * 512], in_=pa[n][:],
                func=mybir.ActivationFunctionType.Tanh, scale=1.0)
            nb = N // out_banks
            if (n + 1) % nb == 0:
                lo, hi = (n + 1 - nb) * 512, (n + 1) * 512
                nc.sync.dma_start(out=y[m * 128:(m + 1) * 128, lo:hi],
                                  in_=o[:, lo:hi])

        # ---- phase 2: finish the split tiles (k >= kh, add partial) ----
        # bank-major: each psum bank's k-loop completes and evacuates under
        # the next bank's matmuls, so banks recycle without WAR bubbles
        for g in range(m_split // 2):
            ms = (2 * g, 2 * g + 1)
            pas = {m: alloc_banks(m) for m in ms}
            os_ = {m: ostage.tile([128, o_loc], out_dt, tag="o", name=f"o{m}")
                   for m in ms}
            for n in range(N):
                for m in ms:
                    for k in range(kh, KP):
                        mm1(pas[m], m, k, n, start=(k == kh),
                            stop=(k == KP - 1))
                for m in ms:
                    evac_bank(m, pas[m], parts[m], os_[m], n, out_banks=2)
            if g < 2:
                flush_deferred((n_h1_def + 1) // 2)  # x h1, for m-tiles 8+
        flush_deferred(len(deferred))

        # ---- remaining m-tiles: single-pass full K, bank-major ----
        for m in range(m_split, M):
            pa = alloc_banks(m)
            o = ostage.tile([128, o_loc], out_dt, tag="o", name=f"o{m}")
            ob = 4 if m == M - 1 else 2
            for n in range(N):
                for k in range(KP):
                    mm1(pa, m, k, n, start=(k == 0), stop=(k == KP - 1))
                evac_bank(m, pa, None, o, n, out_banks=ob,
                          split_last=(m == M - 1 and n == N - 1))

    nc.compile()
    return nc


_NC_CACHE = {}


def _get_nc(key, **kwargs):
    if key not in _NC_CACHE:
        _NC_CACHE[key] = build_nc(**kwargs)
    return _NC_CACHE[key]


def kernel(inputs: np.ndarray, kernel: np.ndarray, bias: np.ndarray,
           _trace: bool = False, _trace_cores=None, **_build_overrides) -> np.ndarray:
    # host-side binarize: ship {-1.0, +1.0} fp8e4 directly (0x38 / 0xB8),
    # exactly matching the reference where(v < 0, -1, 1)
    x8 = np.where(np.asarray(inputs, dtype=np.float32) < 0,
                  np.uint8(0xB8), np.uint8(0x38)).view(ml_dtypes.float8_e4m3)
    w8 = np.where(np.asarray(kernel, dtype=np.float32) < 0,
                  np.uint8(0xB8), np.uint8(0x38)).view(ml_dtypes.float8_e4m3)
    b = np.ascontiguousarray(bias, dtype=np.float32)
    assert x8.shape == (B_FULL, D_FULL) and w8.shape == (D_FULL, O_FULL)

    bias_nonzero = bool(np.any(b != 0))
    key = ("v3", bias_nonzero, tuple(sorted(_build_overrides.items())))
    nc = _get_nc(key, bias_nonzero=bias_nonzero, **_build_overrides)

    in_maps = []
    for i in range(N_CORES):
        r, c = i // C_SHARDS, i % C_SHARDS
        # pre-interleave w for DoubleRow: wire row k*128+p = [w[k*256+p],
        # w[k*256+128+p]] concatenated -> pair DMA is a contiguous row slice
        wsh = w8[:, c * O_LOC:(c + 1) * O_LOC]
        wt = wsh.reshape(D_FULL // 256, 2, 128, O_LOC).transpose(0, 2, 1, 3)
        in_maps.append({
            "x": np.ascontiguousarray(x8[r * B_LOC:(r + 1) * B_LOC, :].T),
            "w": np.ascontiguousarray(wt).reshape(D_FULL // 2, 2 * O_LOC),
            "b": np.ascontiguousarray(b[c * O_LOC:(c + 1) * O_LOC]),
        })

    res = run_bass_kernel_spmd(nc, in_maps, list(range(N_CORES)),
                               trace=_trace, trace_cores=_trace_cores)

    out = np.empty((B_FULL, O_FULL), dtype=np.float32)
    for i in range(N_CORES):
        r, c = i // C_SHARDS, i % C_SHARDS
        out[r * B_LOC:(r + 1) * B_LOC, c * O_LOC:(c + 1) * O_LOC] = \
            np.asarray(res.results[i]["y"]).astype(np.float32)

    if _trace:
        return out, res
    return out

